# revision 1
# baseline (speedup 1.0000x reference)
import numpy as np

# nn_KnowEncoder: hardcoded model dims (from problem spec)
L_, H_, D_ = 16, 16, 64           # num_layers, num_heads, head_dim
G_ = 128                          # conv groups
E_ = 768                          # hidden size
SEG_ = 4                          # E*2L / G / Cin_g segments: 768/192
CIN_G = 192                       # in-channels per group
COUT_G = 256                      # out-channels per group

_DEV = None


def _get_devices():
    global _DEV
    if _DEV is None:
        try:
            import jax
            devs = jax.devices()
            _DEV = devs if devs else []
        except Exception:
            _DEV = []
    return _DEV


def _grouped_gemm(last_hidden, weight):
    """y[b, g*256+o, t] = sum_i lh[b, t, (g%4)*192+i] * w[g, o, i].

    Computed as 4 dense GEMMs (one per channel segment), sharded over
    available neuron cores on the (B*k*n) row axis when jax devices
    exist, else host BLAS.
    """
    Bk, n, E = last_hidden.shape
    # [Bk, n, 4, 192] -> [4, Bk*n, 192]
    lh_segs = np.ascontiguousarray(
        last_hidden.reshape(Bk * n, SEG_, CIN_G).transpose(1, 0, 2))
    # weight [128, 256, 192] -> [32, 4, 256, 192] -> per-seg [4, 32*256, 192]
    W = np.ascontiguousarray(
        weight.reshape(G_ // SEG_, SEG_, COUT_G, CIN_G).transpose(1, 0, 2, 3)
        .reshape(SEG_, (G_ // SEG_) * COUT_G, CIN_G))

    devs = _get_devices()
    if devs:
        try:
            import jax
            import jax.numpy as jnp

            nd = min(len(devs), 8)
            rows = Bk * n            # 4096, divisible by 8
            per = rows // nd

            @jax.jit
            def _mm(a, w):
                # a: [4, per, 192], w: [4, 8192, 192] -> [4, per, 8192]
                return jnp.einsum('spi,soi->spo', a, w,
                                  preferred_element_type=jnp.float32)

            outs = []
            futs = []
            for d in range(nd):
                a_d = jax.device_put(lh_segs[:, d * per:(d + 1) * per, :],
                                     devs[d])
                w_d = jax.device_put(W, devs[d])
                futs.append(_mm(a_d, w_d))
            for f in futs:
                outs.append(np.asarray(f))
            out = np.concatenate(outs, axis=1)   # [4, Bk*n, 8192]
        except Exception:
            out = np.matmul(lh_segs, W.transpose(0, 2, 1))
    else:
        out = np.matmul(lh_segs, W.transpose(0, 2, 1))  # [4, Bk*n, 8192]

    # out[s, b*n+t, m*256+o] -> y[b, (m*4+s)*256+o, t]
    out = out.reshape(SEG_, Bk, n, G_ // SEG_, COUT_G)
    y = out.transpose(1, 3, 0, 4, 2).reshape(Bk, G_ * COUT_G, n)
    return np.ascontiguousarray(y)


def kernel(last_hidden, attention_mask, weight, bias, k):
    last_hidden = np.asarray(last_hidden, np.float32)
    mask = np.asarray(attention_mask)
    weight = np.asarray(weight, np.float32)
    bias = np.asarray(bias, np.float32)
    k = int(k)

    Bk, n, E = last_hidden.shape
    B = Bk // k

    # ---- mean pooling over valid tokens ----
    mask_f = mask.astype(np.float32)
    pooled = ((last_hidden * mask_f[..., None]).sum(1)
              / mask_f.sum(1)[..., None]).astype(np.float32)

    # ---- grouped 1x1 conv as per-segment GEMM ----
    y = _grouped_gemm(last_hidden, weight) + bias[None, :, None]

    # ---- reshape exactly as reference ----
    # [Bk, 32768, n] -> [B, k, L, 2, D, H, n] -> [B, k, L, 2, H, n, D]
    y = y.reshape(B, k, L_, 2, D_, H_, n).transpose(0, 1, 2, 3, 5, 6, 4)
    # -> [B, k*n, L, 2, H, D]
    y = np.ascontiguousarray(y.transpose(0, 1, 5, 2, 3, 4, 6)).reshape(
        B, k * n, L_, 2, H_, D_)

    # ---- ragged gather ----
    lens = mask.sum(1).reshape(B, k)
    Tmax = int(lens.sum(1).max())
    idx = np.zeros((B, Tmax), np.int32)
    valid = np.zeros((B, Tmax), np.int32)
    for i in range(B):
        pos = np.concatenate(
            [j * n + np.arange(lens[i, j]) for j in range(k)])
        idx[i, :pos.size] = pos
        valid[i, :pos.size] = 1

    batch = y[np.arange(B)[:, None], idx]           # [B, Tmax, L, 2, H, D]
    batch = batch * valid[:, :, None, None, None, None].astype(np.float32)
    batch = np.ascontiguousarray(batch.transpose(2, 3, 0, 4, 1, 5))
    # [L, 2, B, H, Tmax, D]
    return pooled, batch, valid.astype(np.int32)
